# revision 12
# baseline (speedup 1.0000x reference)
"""Trainium2 Bass kernel for nn_BaselineAttnDecoder (v3, feature-stationary).

v2 -> v3: shortened softmax chains (qe mask folded into the prod column,
ie mask folded into the psi matmul as rank-17 indicator rows, no
max-subtraction -- scores are bounded ~3), and x-part gate matmuls
precomputed two steps ahead into SBUF (injected later via identity
pairs) so the PE has work during the serial gate-math chain.
"""
import numpy as np
import ml_dtypes

import concourse.bass as bass
import concourse.bacc as bacc
import concourse.mybir as mybir
import concourse.tile as tile
from concourse.masks import make_identity

F32 = mybir.dt.float32
BF16 = mybir.dt.float16
U32 = mybir.dt.uint32
AF = mybir.ActivationFunctionType
ALU = mybir.AluOpType
AX = mybir.AxisListType

D, H, V, K = 300, 512, 8835, 50
L, MAX_LEN, ROUNDS = 20, 21, 10
BS = 160
NCORES = 8
PBS = [128, 32]
BOFF = [0, 128]
IL = 256
VP = 18 * 512
NEG = -30000.0
XR = [128, 128, 65]
K1 = K + 1          # qe contraction incl mask column
KI = K + 18         # psi contraction incl zero row + 16 indicators + const


def bcast_mid(ap, reps):
    return bass.AP(tensor=ap.tensor, offset=ap.offset,
                   ap=[ap.ap[0], [0, reps], ap.ap[1]])


def bcast_in(ap, reps):
    return bass.AP(tensor=ap.tensor, offset=ap.offset,
                   ap=[ap.ap[0], ap.ap[1], [0, reps]])


def regroup(ap, n, sz):
    return bass.AP(tensor=ap.tensor, offset=ap.offset,
                   ap=[ap.ap[0], [sz, n], [1, sz]])


def build_nc():
    nc = bacc.Bacc()

    def din(name, shape, dt):
        return nc.dram_tensor(name, shape, dt, kind="ExternalInput")

    w_gi = din("w_gi", [128, 11, 3 * H], BF16)   # tanh-form scaled
    w_gh = din("w_gh", [128, 4, 3 * H], BF16)
    bhh_n = din("bhh_n", [1, H], BF16)
    w_egi = din("w_egi", [128, 3, 3 * H], BF16)  # direct form
    w_egh = din("w_egh", [128, 4, 3 * H], BF16)
    ebhh_n = din("ebhh_n", [1, H], BF16)
    w_out = din("w_out", [128, 12, D], BF16)
    outb = din("outb", [1, D], BF16)
    w_qk = din("w_qk", [128, 4, K], BF16)
    w_qv = din("w_qv", [128, 4, H], BF16)
    w_ak = din("w_ak", [128, 4, K], BF16)
    akb = din("akb", [1, K], BF16)
    w_ik = din("w_ik", [128, 2, K], BF16)
    w_iv = din("w_iv", [128, 2, H], BF16)
    ivb = din("ivb", [1, H], BF16)
    qvb_c = din("qvb_c", [128, 4], F32)
    img_t = din("img_t", [128, 2, IL], BF16)
    emb_bf = din("emb_bf", [V, D], BF16)
    emb_aug = din("emb_aug", [V, D + 1], F32)
    embt_bf = din("embt_bf", [128, 3, VP], BF16)
    q_idx = din("q_idx", [128, 2 * L], U32)
    a_idx = din("a_idx", [128, 2 * L], U32)
    qe_mask = din("qe_mask", [128, 2, L], F32)
    am_h = din("am_h", [128, BS], BF16)
    ike_h = din("ike_h", [128, IL], BF16)
    ie_mask = din("ie_mask", [128, 2, IL], F32)

    out_o = nc.dram_tensor("out_o", [MAX_LEN, 3, 128, BS], F32,
                           kind="ExternalOutput")

    with tile.TileContext(nc) as tc:
        with (
            tc.tile_pool(name="cw", bufs=1) as cw,
            tc.tile_pool(name="pers", bufs=1) as pers,
            tc.tile_pool(name="wk", bufs=2) as wk,
            tc.tile_pool(name="st", bufs=1) as st,
            tc.tile_pool(name="pg", bufs=1, space="PSUM") as pgp,
            tc.tile_pool(name="ph", bufs=1, space="PSUM") as php,
        ):
            def load(pool, t, dt):
                s = pool.tile(list(t.shape), dt, name=t.name + "_sb")
                nc.sync.dma_start(s[:], t[:])
                return s

            s_qk = load(cw, w_qk, BF16)
            s_qv = load(cw, w_qv, BF16)
            s_ak = load(cw, w_ak, BF16)
            s_ik = load(cw, w_ik, BF16)
            s_iv = load(cw, w_iv, BF16)
            s_ivb = load(cw, ivb, BF16)
            s_imgt = load(cw, img_t, BF16)
            s_bhh = load(cw, bhh_n, BF16)
            s_ebhh = load(cw, ebhh_n, BF16)
            s_outb = load(cw, outb, BF16)
            s_akb = load(cw, akb, BF16)
            s_qvb = load(cw, qvb_c, F32)
            s_qidx = load(cw, q_idx, U32)
            s_aidx = load(cw, a_idx, U32)
            s_qem = load(cw, qe_mask, F32)
            s_iem = load(cw, ie_mask, F32)

            ident_bf = cw.tile([128, 128], BF16)
            make_identity(nc, ident_bf[:])
            ident_f32 = cw.tile([128, 128], F32)
            make_identity(nc, ident_f32[:])
            ones_bf = cw.tile([1, 192], BF16)
            nc.vector.memset(ones_bf[:], 1.0)
            sid4 = cw.tile([128, 32], BF16)
            for g4 in range(4):
                nc.vector.tensor_copy(sid4[32 * g4:32 * (g4 + 1), :],
                                      ident_bf[0:32, 0:32])
            iota8 = cw.tile([128, 8], F32)
            nc.gpsimd.iota(iota8[:], pattern=[[1, 8]], base=0, channel_multiplier=0,
                           allow_small_or_imprecise_dtypes=True)

            PG = pgp.tile([128, 4, 512], F32, name="PG")
            PH = php.tile([128, 4, 512], F32, name="PH")

            def trp_slot(bank, lo, hi):
                return PH[:, bank, lo:hi].bitcast(BF16)

            hD = pers.tile([128, 4, BS], BF16, name="hD")
            qk_b0 = pers.tile([128, L, K1], BF16)
            qk_b1 = pers.tile([128, L, K1], BF16)
            qkbs = [qk_b0, qk_b1]
            qv_b0 = pers.tile([128, L, H], BF16)
            qv_p1 = pers.tile([128, 5, H], BF16)
            ivv = pers.tile([128, 2, H], BF16)
            ikt = pers.tile([128, IL], BF16)
            nc.sync.dma_start(ikt[:], ike_h[:])
            qcT = pers.tile([128, 4, BS], BF16)
            icT = pers.tile([128, 4, BS], BF16)
            dec20 = pers.tile([128, 3, BS], BF16)

            nc.vector.memset(hD[:], 0.0)
            nc.vector.memset(dec20[32:64, 2, :], 0.0)
            nc.vector.memset(dec20[64:65, 2, :], 1.0)

            def tr(dst_sb_ap, src_sb_ap, pb, w, pt_ap, eng=None):
                nc.tensor.transpose(pt_ap[:w, :pb], src_sb_ap,
                                    ident_bf[:pb, :pb])
                (eng or nc.vector).tensor_copy(dst_sb_ap, pt_ap[:w, :pb])

            def tr_add(dst_sb_ap, src_sb_ap, bias_ap, pb, w, pt_ap):
                nc.tensor.transpose(pt_ap[:w, :pb], src_sb_ap,
                                    ident_bf[:pb, :pb])
                nc.vector.tensor_scalar_add(dst_sb_ap, pt_ap[:w, :pb], bias_ap)

            def fetch_x(idx_sb, t, slots):
                xt = wk.tile([128, 3, BS], BF16, tag="xt", bufs=4, name="xt")
                nc.vector.memset(xt[32:64, 2, :], 0.0)
                nc.vector.memset(xt[64:65, 2, :], 1.0)
                i = 0
                for c, (pb, off) in enumerate(zip(PBS, BOFF)):
                    g = wk.tile([128, D], BF16, tag="gath", bufs=4, name="g")
                    nc.gpsimd.indirect_dma_start(
                        out=g[:pb], out_offset=None, in_=emb_bf[:],
                        in_offset=bass.IndirectOffsetOnAxis(
                            ap=idx_sb[:pb, 2 * t + c:2 * t + c + 1], axis=0))
                    for kt in range(3):
                        w = 128 if kt < 2 else D - 256
                        tr(xt[:w, kt, off:off + pb], g[:pb, kt * 128:kt * 128 + w],
                           pb, w, slots[i % len(slots)])
                        i += 1
                return xt

            def emit_group(ps_ap, pairs):
                n = len(pairs)
                for i, (lh, rh) in enumerate(pairs):
                    nc.tensor.matmul(ps_ap, lh, rh, start=(i == 0), stop=(i == n - 1))

            def xh_round(wt, gbase, xt, dst, sl0):
                for j in range(4):
                    gc = slice(gbase + j * 128, gbase + (j + 1) * 128)
                    pairs = [(wt[:XR[kt], kt, gc], xt[:XR[kt], kt, :])
                             for kt in range(3)]
                    emit_group(PG[:, j, sl0:sl0 + BS], pairs)
                nc.vector.tensor_copy(dst[:], PG[:, :, sl0:sl0 + BS])

            # ---------- image projections ----------
            for mt in range(2):
                psv = PG[:, mt, 0:512]
                emit_group(psv, [(s_imgt[:, kt, mt * 128:(mt + 1) * 128],
                                  s_iv[:, kt, :]) for kt in range(2)])
                nc.scalar.copy(ivv[:, mt, :], psv)
            psik = PG[:K, 2, 0:IL]
            emit_group(psik, [(s_ik[:, kt, :K], s_imgt[:, kt, :])
                              for kt in range(2)])
            nc.vector.tensor_copy(ikt[:K, :], psik)

            # =================== encoder ===================
            with tc.tile_pool(name="qp", bufs=1) as qp:
                s_egi = load(qp, w_egi, BF16)
                s_egh = load(qp, w_egh, BF16)
                hE = qp.tile([128, 4, BS], BF16, name="hE")
                nc.vector.memset(hE[:], 0.0)
                xh_er = qp.tile([128, 3, 4, BS], BF16, name="xh_er")
                xh_ez = qp.tile([128, 3, 4, BS], BF16, name="xh_ez")
                xh_en = qp.tile([128, 3, 4, BS], F32, name="xh_en")
                enc_fsl = [trp_slot(0, 256, 336), trp_slot(1, 256, 336)]

                def enc_xh(tt):
                    xt = fetch_x(s_qidx, tt, enc_fsl)
                    r3 = tt % 3
                    xh_round(s_egi, 0, xt, xh_er[:, r3], 320)
                    xh_round(s_egi, H, xt, xh_ez[:, r3], 320)
                    xh_round(s_egi, 2 * H, xt, xh_en[:, r3], 320)

                enc_xh(0)
                enc_xh(1)
                for t in range(L):
                    r3 = t % 3
                    for j in range(4):
                        mc = slice(j * 128, (j + 1) * 128)
                        zc = slice(H + j * 128, H + (j + 1) * 128)
                        nxc = slice(2 * H + j * 128, 2 * H + (j + 1) * 128)
                        pairs = [(ident_bf[:], xh_er[:, r3, j, :])]
                        pairs += [(s_egh[:, kt, mc], hE[:, kt, :]) for kt in range(4)]
                        emit_group(PG[:, j, 0:BS], pairs)
                        pairs = [(ident_bf[:], xh_ez[:, r3, j, :])]
                        pairs += [(s_egh[:, kt, zc], hE[:, kt, :]) for kt in range(4)]
                        emit_group(PG[:, j, 160:160 + BS], pairs)
                        pairs = [(s_egh[:, kt, nxc], hE[:, kt, :]) for kt in range(4)]
                        pairs.append((s_ebhh[:, j * 128:(j + 1) * 128],
                                      ones_bf[:, :BS]))
                        emit_group(PH[:, j, 0:BS], pairs)
                    if t + 2 < L:
                        enc_xh(t + 2)
                    rz = st.tile([128, 4, 320], F32, tag="rz", name="rz")
                    nc.scalar.activation(rz[:], PG[:, :, 0:320], AF.Sigmoid)
                    t1 = st.tile([128, 4, BS], F32, tag="t1", name="t1")
                    nc.gpsimd.tensor_mul(t1[:], rz[:, :, 0:BS], PH[:, :, 0:BS])
                    cc = st.tile([128, 4, BS], F32, tag="cc", name="cc")
                    nc.vector.tensor_add(cc[:], t1[:], xh_en[:, r3])
                    n4 = st.tile([128, 4, BS], F32, tag="n4", name="n4")
                    nc.scalar.activation(n4[:], cc[:], AF.Tanh)
                    d4 = st.tile([128, 4, BS], F32, tag="d4", name="d4")
                    nc.gpsimd.tensor_sub(d4[:], hE[:], n4[:])
                    e4 = st.tile([128, 4, BS], F32, tag="e4", name="e4")
                    nc.gpsimd.tensor_mul(e4[:], rz[:, :, 160:160 + BS], d4[:])
                    nc.gpsimd.tensor_add(hE[:], n4[:], e4[:])
                    for bt, (pb, off) in enumerate(zip(PBS, BOFF)):
                        psk = PH[:pb, bt, 160:160 + K]
                        emit_group(psk, [(hE[:, kt, off:off + pb], s_qk[:, kt, :])
                                         for kt in range(4)])
                        nc.scalar.copy(qkbs[bt][:pb, t, 0:K], psk)
                        psv = PH[:pb, 2 + bt, 0:512]
                        emit_group(psv, [(hE[:, kt, off:off + pb], s_qv[:, kt, :])
                                         for kt in range(4)])
                        if bt == 0:
                            nc.scalar.copy(qv_b0[:pb, t, :], psv)
                        else:
                            g4 = t % 4
                            nc.scalar.copy(qv_p1[32 * g4:32 * (g4 + 1), t // 4, :],
                                           psv)

            # =================== decoder ===================
            with tc.tile_pool(name="lg", bufs=1) as lg:
                s_gi = load(lg, w_gi, BF16)
                s_gh = load(lg, w_gh, BF16)
                s_out = load(lg, w_out, BF16)
                o19T = lg.tile([128, 3, BS], BF16)
                nc.vector.memset(o19T[32:64, 2, :], 0.0)
                nc.vector.memset(o19T[64:65, 2, :], 1.0)
                o19_0 = lg.tile([128, D], F32)
                o19_1 = lg.tile([128, D], F32)
                o19_sb = [o19_0, o19_1]
                logit_sb = lg.tile([128, 8840], BF16)
                xh_r = lg.tile([128, 3, 4, BS], BF16, name="xh_r")
                xh_z = lg.tile([128, 3, 4, BS], BF16, name="xh_z")
                xh_n = lg.tile([128, 3, 4, BS], BF16, name="xh_n")
                dec_fsl = [trp_slot(2, 80, 160), trp_slot(3, 80, 160)]

                for c, (pb, off) in enumerate(zip(PBS, BOFF)):
                    nc.vector.tensor_copy(qkbs[c][:pb, :, K:K1],
                                          s_qem[:pb, c, :].unsqueeze(-1))
                aT_sb = lg.tile([128, BS], BF16, name="aT_pers")
                nc.sync.dma_start(aT_sb[:], am_h[:])

                def dec_xh(tt):
                    xt = fetch_x(s_aidx, tt, dec_fsl)
                    r3 = tt % 3
                    xh_round(s_gi, 0, xt, xh_r[:, r3], 160)
                    xh_round(s_gi, H, xt, xh_z[:, r3], 160)
                    xh_round(s_gi, 2 * H, xt, xh_n[:, r3], 160)

                dec_xh(0)
                dec_xh(1)
                for t in range(MAX_LEN):
                    r3 = t % 3
                    use_dec20 = (t == MAX_LEN - 1)
                    pa = PH[:K, 0, 160:160 + BS]
                    pairs = [(s_ak[:, kt, :], hD[:, kt, :]) for kt in range(4)]
                    pairs.append((s_akb[:], ones_bf[:, :BS]))
                    emit_group(pa, pairs)
                    nc.vector.tensor_copy(aT_sb[:K, :], pa)
                    for j in range(4):
                        nxc = slice(2 * H + j * 128, 2 * H + (j + 1) * 128)
                        pairs = [(s_gh[:, kt, nxc], hD[:, kt, :]) for kt in range(4)]
                        pairs.append((s_bhh[:, j * 128:(j + 1) * 128],
                                      ones_bf[:, :BS]))
                        if j < 3:
                            emit_group(PH[:, 1, 160 * j:160 * j + BS], pairs)
                        else:
                            emit_group(PH[:, 0, 320:320 + BS], pairs)
                    nh_sb = st.tile([128, 4, BS], F32, tag="nh", name="nh_sb")
                    nc.vector.tensor_copy(nh_sb[:, 0:3, :],
                                          regroup(PH[:, 1, 0:480], 3, BS))
                    nc.vector.tensor_copy(nh_sb[:, 3, :], PH[:, 0, 320:320 + BS])

                    a_b = st.tile([128, 2, K1], BF16, tag="ab", name="a_b")
                    for c, (pb, off) in enumerate(zip(PBS, BOFF)):
                        tr(a_b[:pb, c, :], aT_sb[:K1, off:off + pb], K1, pb,
                           trp_slot(2, 0, 80), eng=nc.gpsimd)
                    psis = []
                    for c, (pb, off) in enumerate(zip(PBS, BOFF)):
                        psi = PH[:pb, 2 + c, 160:160 + IL]
                        nc.tensor.matmul(psi, aT_sb[:KI, off:off + pb],
                                         ikt[:KI, :], start=True, stop=True)
                        psis.append(psi)

                    qw_bf = st.tile([128, 2, L], BF16, tag="qw", name="qw_bf")
                    for c, (pb, off) in enumerate(zip(PBS, BOFF)):
                        eng = nc.vector if c == 0 else nc.gpsimd
                        prod = wk.tile([128, L, K1], BF16, tag="prod", bufs=1,
                                       name="prod")
                        eng.tensor_mul(prod[:pb], qkbs[c][:pb],
                                       bcast_mid(a_b[:pb, c, :], L))
                        qe = st.tile([128, L], F32, tag="qe" + str(c), name="qe")
                        nc.vector.tensor_reduce(qe[:pb], prod[:pb], axis=AX.X,
                                                op=ALU.add)
                        ew = st.tile([128, L], F32, tag="ew" + str(c), name="ew")
                        ssum = st.tile([128, 1], F32, tag="ss" + str(c), name="ssum")
                        nc.scalar.activation(ew[:pb], qe[:pb], AF.Exp,
                                             scale=1.0, accum_out=ssum[:pb])
                        rs = st.tile([128, 1], F32, tag="rs" + str(c), name="rs")
                        nc.vector.reciprocal(rs[:pb], ssum[:pb])
                        nc.vector.tensor_scalar_mul(qw_bf[:pb, c, :], ew[:pb],
                                                    rs[:pb])

                    iwT = st.tile([128, 2, BS], BF16, tag="iwT", name="iwT")
                    for c, (pb, off) in enumerate(zip(PBS, BOFF)):
                        eng = nc.vector if c == 0 else nc.gpsimd
                        iem = st.tile([128, IL], F32, tag="iem" + str(c), name="iem")
                        eng.tensor_add(iem[:pb], psis[c], s_iem[:pb, c, :])
                        ewi = st.tile([128, IL], F32, tag="ewi" + str(c), name="ewi")
                        ssi = st.tile([128, 1], F32, tag="ssi" + str(c), name="ssi")
                        nc.scalar.activation(ewi[:pb], iem[:pb], AF.Exp,
                                             scale=1.0, accum_out=ssi[:pb])
                        rsi = st.tile([128, 1], F32, tag="rsi" + str(c), name="rsi")
                        nc.vector.reciprocal(rsi[:pb], ssi[:pb])
                        iwb = st.tile([128, IL], BF16, tag="iwb" + str(c),
                                      name="iwb")
                        eng.tensor_scalar_mul(iwb[:pb], ewi[:pb], rsi[:pb])
                        for cc2 in range(2):
                            tr(iwT[:, cc2, off:off + pb],
                               iwb[:pb, cc2 * 128:(cc2 + 1) * 128], pb, 128,
                               trp_slot(3, 0, 80))

                    ic_slots = [PH[:, 0, 160:160 + BS], PH[:, 1, 0:BS],
                                PH[:, 1, 160:160 + BS], PH[:, 1, 320:320 + BS]]
                    for j in range(4):
                        pairs = [(ivv[:, kt, j * 128:(j + 1) * 128],
                                  iwT[:, kt, :]) for kt in range(2)]
                        pairs.append((s_ivb[:, j * 128:(j + 1) * 128],
                                      ones_bf[:, :BS]))
                        emit_group(ic_slots[j], pairs)
                    nc.vector.tensor_copy(icT[:, 0, :], PH[:, 0, 160:160 + BS])
                    nc.vector.tensor_copy(icT[:, 1:4, :],
                                          regroup(PH[:, 1, 0:480], 3, BS))

                    qcb = st.tile([128, 2, 512], BF16, tag="qcb", name="qcb")
                    dg = wk.tile([128, L, 128], BF16, tag="diag", bufs=1, name="dg")
                    hl = L // 2
                    ibh = ident_bf[:128, :128]
                    ident_h = bass.AP(tensor=ibh.tensor, offset=ibh.offset,
                                      ap=[ibh.ap[0], [0, hl], ibh.ap[1]])
                    nc.gpsimd.tensor_mul(dg[:, :hl, :],
                                         bcast_in(qw_bf[:, 0, :hl], 128), ident_h)
                    nc.vector.tensor_mul(dg[:, hl:, :],
                                         bcast_in(qw_bf[:, 0, hl:], 128), ident_h)
                    psqc0 = PH[:, 2, 0:512]
                    for l in range(L):
                        nc.tensor.matmul(psqc0, dg[:, l, :], qv_b0[:, l, :],
                                         start=(l == 0), stop=(l == L - 1))
                    nc.scalar.copy(qcb[:, 0, :], psqc0)
                    qw_pk = st.tile([128, 5], BF16, tag="qpk", name="qw_pk")
                    for g4 in range(4):
                        nc.vector.tensor_copy(qw_pk[32 * g4:32 * (g4 + 1), :],
                                              qw_bf[0:32, 1, g4:L:4])
                    dg1 = wk.tile([128, 5, 32], BF16, tag="dg1", name="dg1")
                    sid_b = bass.AP(tensor=sid4.tensor, offset=sid4[:, :].offset,
                                    ap=[sid4[:, :].ap[0], [0, 5], sid4[:, :].ap[1]])
                    nc.vector.tensor_mul(dg1[:, :, :], bcast_in(qw_pk[:, :], 32),
                                         sid_b)
                    psqc1 = PH[:32, 3, 0:512]
                    for c5 in range(5):
                        nc.tensor.matmul(psqc1, dg1[:, c5, :], qv_p1[:, c5, :],
                                         start=(c5 == 0), stop=(c5 == 4))
                    nc.scalar.copy(qcb[:32, 1, :], psqc1)
                    for c, (pb, off) in enumerate(zip(PBS, BOFF)):
                        for kt in range(4):
                            tr_add(qcT[:, kt, off:off + pb],
                                   qcb[:pb, c, kt * 128:(kt + 1) * 128],
                                   s_qvb[:, kt:kt + 1], pb, 128,
                                   trp_slot(2, 0, 80) if c == 0
                                   else trp_slot(3, 0, 80))

                    # late gate groups
                    for j in range(4):
                        mc = slice(j * 128, (j + 1) * 128)
                        if use_dec20:
                            pairs = [(s_gi[:XR[kt], kt, mc], dec20[:XR[kt], kt, :])
                                     for kt in range(3)]
                        else:
                            pairs = [(ident_bf[:], xh_r[:, r3, j, :])]
                        pairs += [(s_gh[:, kt, mc], hD[:, kt, :]) for kt in range(4)]
                        pairs += [(s_gi[:, 3 + k, mc], qcT[:, k, :]) for k in range(4)]
                        pairs += [(s_gi[:, 7 + k, mc], icT[:, k, :]) for k in range(4)]
                        emit_group(PG[:, j, 0:BS], pairs)
                    for j in range(4):
                        zc = slice(H + j * 128, H + (j + 1) * 128)
                        if use_dec20:
                            pairs = [(s_gi[:XR[kt], kt, zc], dec20[:XR[kt], kt, :])
                                     for kt in range(3)]
                        else:
                            pairs = [(ident_bf[:], xh_z[:, r3, j, :])]
                        pairs += [(s_gh[:, kt, zc], hD[:, kt, :]) for kt in range(4)]
                        pairs += [(s_gi[:, 3 + k, zc], qcT[:, k, :]) for k in range(4)]
                        pairs += [(s_gi[:, 7 + k, zc], icT[:, k, :]) for k in range(4)]
                        emit_group(PH[:, j, 0:BS], pairs)
                    for j in range(4):
                        nxc = slice(2 * H + j * 128, 2 * H + (j + 1) * 128)
                        if use_dec20:
                            pairs = [(s_gi[:XR[kt], kt, nxc],
                                      dec20[:XR[kt], kt, :]) for kt in range(3)]
                        else:
                            pairs = [(ident_bf[:], xh_n[:, r3, j, :])]
                        pairs += [(s_gi[:, 3 + k, nxc], qcT[:, k, :]) for k in range(4)]
                        pairs += [(s_gi[:, 7 + k, nxc], icT[:, k, :]) for k in range(4)]
                        emit_group(PG[:, j, 320:320 + BS], pairs)

                    if t + 2 < L:
                        dec_xh(t + 2)

                    trz = st.tile([128, 4, BS], F32, tag="trz", name="trz")
                    tzz = st.tile([128, 4, BS], F32, tag="tzz", name="tzz")
                    nc.scalar.activation(trz[:], PG[:, :, 0:BS], AF.Tanh)
                    nc.scalar.activation(tzz[:], PH[:, :, 0:BS], AF.Tanh)
                    aa = st.tile([128, 4, BS], F32, tag="aa", name="aa")
                    nc.gpsimd.tensor_mul(aa[:], trz[:], nh_sb[:])
                    bb = st.tile([128, 4, BS], F32, tag="bb", name="bb")
                    nc.gpsimd.tensor_add(bb[:], aa[:], nh_sb[:])
                    cc = st.tile([128, 4, BS], F32, tag="cc", name="cc")
                    nc.vector.tensor_add(cc[:], bb[:], PG[:, :, 320:320 + BS])
                    n4 = st.tile([128, 4, BS], F32, tag="n4", name="n4")
                    nc.scalar.activation(n4[:], cc[:], AF.Tanh, scale=0.5)
                    z4 = st.tile([128, 4, BS], F32, tag="z4", name="z4")
                    nc.vector.tensor_scalar(out=z4[:], in0=tzz[:], scalar1=0.5,
                                            scalar2=0.5, op0=ALU.mult,
                                            op1=ALU.add)
                    d4 = st.tile([128, 4, BS], F32, tag="d4", name="d4")
                    nc.gpsimd.tensor_sub(d4[:], hD[:], n4[:])
                    e4 = st.tile([128, 4, BS], F32, tag="e4", name="e4")
                    nc.gpsimd.tensor_mul(e4[:], z4[:], d4[:])
                    nc.gpsimd.tensor_add(hD[:], n4[:], e4[:])

                    oT = st.tile([128, 3, BS], F32, tag="oT", bufs=2, name="oT")
                    po_slots = [PH[:, 2, 160:160 + BS], PH[:, 3, 160:160 + BS],
                                PH[:D - 256, 2, 320:320 + BS]]
                    for j in range(3):
                        w = 128 if j < 2 else D - 256
                        mc = slice(j * 128, j * 128 + w)
                        po = po_slots[j]
                        pairs = [(s_out[:, k, mc], hD[:, k, :]) for k in range(4)]
                        pairs += [(s_out[:, 4 + k, mc], qcT[:, k, :])
                                  for k in range(4)]
                        pairs += [(s_out[:, 8 + k, mc], icT[:, k, :])
                                  for k in range(4)]
                        pairs.append((s_outb[:, mc], ones_bf[:, :BS]))
                        emit_group(po, pairs)
                        nc.scalar.copy(oT[:w, j, :], po)
                        nc.sync.dma_start(out_o[t, j, :w, :], oT[:w, j, :])

                    if t == MAX_LEN - 2:
                        nc.vector.tensor_copy(o19T[:, 0:2, :], oT[:, 0:2, :])
                        nc.vector.tensor_copy(o19T[:D - 256, 2, :],
                                              oT[:D - 256, 2, :])
                        pt32 = PH[:, 1, 160:288]
                        for c, (pb, off) in enumerate(zip(PBS, BOFF)):
                            for kt in range(3):
                                w = 128 if kt < 2 else D - 256
                                nc.tensor.transpose(pt32[:pb, :w],
                                                    oT[:w, kt, off:off + pb],
                                                    ident_f32[:w, :w])
                                nc.vector.tensor_copy(
                                    o19_sb[c][:pb, kt * 128:kt * 128 + w],
                                    pt32[:pb, :w])
                        for c, (pb, off) in enumerate(zip(PBS, BOFF)):
                            for nci in range(18):
                                ncw = 512 if nci < 17 else V - 17 * 512
                                rhs = wk.tile([128, 3, 512], BF16, tag="lrhs",
                                              bufs=2, name="rhs")
                                for kt in range(3):
                                    nr = 128 if kt < 2 else 65
                                    nc.sync.dma_start(
                                        rhs[:nr, kt, :ncw],
                                        embt_bf[:nr, kt,
                                                nci * 512:nci * 512 + ncw])
                                psl = PH[:pb, 2 + (nci % 2), 0:ncw]
                                pairs = []
                                for kt in range(3):
                                    nr = 128 if kt < 2 else 65
                                    pairs.append((o19T[:nr, kt, off:off + pb],
                                                  rhs[:nr, kt, :ncw]))
                                emit_group(psl, pairs)
                                nc.scalar.copy(
                                    logit_sb[:pb, nci * 512:nci * 512 + ncw],
                                    psl)
                            if c == 0:
                                nc.vector.memset(logit_sb[:, V:], -60000.0)
                            mx8 = st.tile([128, 8], BF16, name="mx8")
                            nc.vector.max(mx8[:pb], logit_sb[:pb])
                            ix8 = st.tile([128, 8], U32, name="ix8")
                            nc.vector.max_index(ix8[:pb], mx8[:pb], logit_sb[:pb])
                            scores = st.tile([128, 8], F32, name="scores")
                            for jj in range(8):
                                g8 = wk.tile([128, D + 1], F32, tag="gath8",
                                             name="g8")
                                nc.gpsimd.indirect_dma_start(
                                    out=g8[:pb], out_offset=None, in_=emb_aug[:],
                                    in_offset=bass.IndirectOffsetOnAxis(
                                        ap=ix8[:pb, jj:jj + 1], axis=0))
                                pr = wk.tile([128, D], F32, tag="pr8", name="pr")
                                nc.vector.tensor_mul(pr[:pb], o19_sb[c][:pb],
                                                     g8[:pb, :D])
                                sj = st.tile([128, 1], F32, name="sj")
                                nc.vector.tensor_reduce(sj[:pb], pr[:pb],
                                                        axis=AX.X, op=ALU.add)
                                nc.vector.tensor_add(scores[:pb, jj:jj + 1],
                                                     sj[:pb], g8[:pb, D:D + 1])
                            m1 = st.tile([128, 8], F32, name="m1")
                            nc.vector.max(m1[:pb], scores[:pb])
                            j1 = st.tile([128, 8], U32, name="j1")
                            nc.vector.max_index(j1[:pb], m1[:pb], scores[:pb])
                            j1f = st.tile([128, 1], F32, name="j1f")
                            nc.vector.tensor_copy(j1f[:pb], j1[:pb, 0:1])
                            oh = st.tile([128, 8], F32, name="oh")
                            nc.vector.tensor_scalar(out=oh[:pb], in0=iota8[:pb],
                                                    scalar1=j1f[:pb],
                                                    scalar2=None,
                                                    op0=ALU.is_equal)
                            ix8f = st.tile([128, 8], F32, name="ix8f")
                            nc.vector.tensor_copy(ix8f[:pb], ix8[:pb])
                            nc.vector.tensor_mul(ix8f[:pb], oh[:pb], ix8f[:pb])
                            vsum = st.tile([128, 1], F32, name="vsum")
                            nc.vector.tensor_reduce(vsum[:pb], ix8f[:pb],
                                                    axis=AX.X, op=ALU.add)
                            vidx = st.tile([128, 1], U32, name="vidx")
                            nc.vector.tensor_copy(vidx[:pb], vsum[:pb])
                            gm = wk.tile([128, D], BF16, tag="gath", bufs=4,
                                         name="gm")
                            nc.gpsimd.indirect_dma_start(
                                out=gm[:pb], out_offset=None, in_=emb_bf[:],
                                in_offset=bass.IndirectOffsetOnAxis(
                                    ap=vidx[:pb, 0:1], axis=0))
                            for kt in range(3):
                                w = 128 if kt < 2 else D - 256
                                tr(dec20[:w, kt, off:off + pb],
                                   gm[:pb, kt * 128:kt * 128 + w], pb, w,
                                   trp_slot(2, 0, 80))

    nc.compile()
    return nc


_NC_CACHE = None


def _get_nc():
    global _NC_CACHE
    if _NC_CACHE is None:
        _NC_CACHE = build_nc()
    return _NC_CACHE


def _pad_tiles(a, ntiles):
    rows, cols = a.shape
    out = np.zeros((128 * ntiles, cols), a.dtype)
    out[:rows] = a
    return np.ascontiguousarray(out.reshape(ntiles, 128, cols).transpose(1, 0, 2))


def _prep_shared(inputs):
    bf = np.float16
    f32 = np.float32
    eW = np.asarray(inputs["embed_W"], f32)
    d = {}
    wih = np.asarray(inputs["dec_W_ih"], f32)
    bih = np.asarray(inputs["dec_b_ih"], f32)
    bhh = np.asarray(inputs["dec_b_hh"], f32)
    gi = np.zeros((128 * 11, 3 * H), f32)
    gi[0:D] = wih[:, 0:D].T
    gi[320] = bih + np.concatenate([bhh[:2 * H], np.zeros(H, f32)])
    gi[384:384 + H] = wih[:, D:D + H].T
    gi[896:896 + H] = wih[:, D + H:].T
    gi[:, 0:2 * H] *= 0.5
    gi[:, 2 * H:] *= 2.0
    d["w_gi"] = _pad_tiles(gi.astype(bf), 11)
    gh = np.asarray(inputs["dec_W_hh"], f32).T.copy()
    gh[:, 0:2 * H] *= 0.5
    d["w_gh"] = _pad_tiles(gh.astype(bf), 4)
    d["bhh_n"] = np.ascontiguousarray(bhh[2 * H:].astype(bf)[None, :])
    ewih = np.asarray(inputs["enc_W_ih"], f32)
    ebih = np.asarray(inputs["enc_b_ih"], f32)
    ebhh = np.asarray(inputs["enc_b_hh"], f32)
    egi = np.zeros((128 * 3, 3 * H), f32)
    egi[0:D] = ewih[:, :D].T
    egi[320] = ebih + np.concatenate([ebhh[:2 * H], np.zeros(H, f32)])
    d["w_egi"] = _pad_tiles(egi.astype(bf), 3)
    d["w_egh"] = _pad_tiles(np.asarray(inputs["enc_W_hh"], f32).T.astype(bf), 4)
    d["ebhh_n"] = np.ascontiguousarray(ebhh[2 * H:].astype(bf)[None, :])
    d["w_out"] = _pad_tiles(np.asarray(inputs["out_W"], f32).T.astype(bf), 12)
    d["outb"] = np.ascontiguousarray(
        np.asarray(inputs["out_b"], f32).astype(bf)[None, :])
    d["w_qk"] = _pad_tiles(np.asarray(inputs["qk_W"], f32).T.astype(bf), 4)
    d["w_qv"] = _pad_tiles(np.asarray(inputs["qv_W"], f32).T.astype(bf), 4)
    d["qvb_c"] = np.ascontiguousarray(
        np.asarray(inputs["qv_b"], f32).reshape(4, 128).T)
    d["w_ak"] = _pad_tiles(np.asarray(inputs["ak_W"], f32).T.astype(bf), 4)
    d["akb"] = np.ascontiguousarray(
        np.asarray(inputs["ak_b"], f32).astype(bf)[None, :])
    d["w_ik"] = _pad_tiles(np.asarray(inputs["ik_W"], f32).T.astype(bf), 2)
    d["w_iv"] = _pad_tiles(np.asarray(inputs["iv_W"], f32).T.astype(bf), 2)
    d["ivb"] = np.ascontiguousarray(
        np.asarray(inputs["iv_b"], f32).astype(bf)[None, :])
    d["emb_bf"] = eW.astype(bf)
    wd_b = np.asarray(inputs["wd_b"], f32)
    d["emb_aug"] = np.ascontiguousarray(np.concatenate([eW, wd_b[:, None]], 1))
    aug = np.zeros((128 * 3, VP), f32)
    aug[:D, :V] = eW.T
    aug[320, :V] = wd_b
    d["embt_bf"] = _pad_tiles(aug.astype(bf), 3)
    am = np.zeros((128, BS), f32)
    am[K, :] = 1.0
    for b in range(BS):
        am[51 + b // ROUNDS, b] = 1.0
    am[67, :] = 1.0
    d["am_h"] = am.astype(bf)
    ike = np.zeros((128, IL), f32)
    for i in range(16):
        ike[51 + i, 16 * i:16 * (i + 1)] = -NEG
    ike[67, :] = NEG
    d["ike_h"] = ike.astype(bf)
    return d


def _idx_cols(seq_rows):
    out = np.zeros((128, 2 * L), np.uint32)
    for t in range(L):
        out[:, 2 * t] = seq_rows[0:128, t]
        out[:32, 2 * t + 1] = seq_rows[128:160, t]
    return out


def _build_maps(inputs, shared):
    f32 = np.float32
    bf = np.float16
    ques = np.asarray(inputs["ques_seqs"]).astype(np.uint32)
    ans = np.asarray(inputs["ans_seqs"]).astype(np.uint32)
    qlens = np.asarray(inputs["ques_lens"]).astype(np.int64)
    img = np.asarray(inputs["img_seqs"], f32)
    maps = []
    for s in range(NCORES):
        m = dict(shared)
        r0 = s * BS
        m["q_idx"] = _idx_cols(ques[r0:r0 + BS, :L])
        m["a_idx"] = _idx_cols(ans[r0:r0 + BS, :L])
        qm = np.full((128, 2, L), NEG, f32)
        lens = qlens[r0:r0 + BS]
        for bt, (pb, off) in enumerate(zip(PBS, BOFF)):
            for b in range(pb):
                qm[b, bt, :lens[off + b]] = 0.0
        m["qe_mask"] = qm
        im = np.full((128, 2, IL), NEG, f32)
        for bt, (pb, off) in enumerate(zip(PBS, BOFF)):
            for b in range(pb):
                gimg = (off + b) // ROUNDS
                im[b, bt, gimg * 16:(gimg + 1) * 16] = 0.0
        m["ie_mask"] = im
        imgs = img[s * 16:(s + 1) * 16].reshape(IL, 256)
        it = np.zeros((128 * 2, IL), f32)
        it[:256] = imgs.T
        m["img_t"] = np.ascontiguousarray(
            it.reshape(2, 128, IL).transpose(1, 0, 2)).astype(bf)
        maps.append(m)
    return maps


def kernel(**inputs):
    nc = _get_nc()
    shared = _prep_shared(inputs)
    in_maps = _build_maps(inputs, shared)
    from concourse.bass_utils import run_bass_kernel_spmd
    res = run_bass_kernel_spmd(nc, in_maps, core_ids=list(range(NCORES)))
    outs = []
    for s in range(NCORES):
        o = np.asarray(res.results[s]["out_o"])  # [21, 3, 128, 160]
        o = o.reshape(MAX_LEN, 3 * 128, BS)[:, :D, :]
        outs.append(np.ascontiguousarray(o.transpose(2, 0, 1)))
    return np.concatenate(outs, 0).astype(np.float32)


# revision 13
# speedup vs baseline: 1.0690x; 1.0690x over previous
"""Trainium2 Bass kernel for nn_BaselineAttnDecoder (v3, feature-stationary).

v2 -> v3: shortened softmax chains (qe mask folded into the prod column,
ie mask folded into the psi matmul as rank-17 indicator rows, no
max-subtraction -- scores are bounded ~3), and x-part gate matmuls
precomputed two steps ahead into SBUF (injected later via identity
pairs) so the PE has work during the serial gate-math chain.
"""
import numpy as np
import ml_dtypes

import concourse.bass as bass
import concourse.bacc as bacc
import concourse.mybir as mybir
import concourse.tile as tile
from concourse.masks import make_identity

F32 = mybir.dt.float32
BF16 = mybir.dt.float16
U32 = mybir.dt.uint32
AF = mybir.ActivationFunctionType
ALU = mybir.AluOpType
AX = mybir.AxisListType

D, H, V, K = 300, 512, 8835, 50
L, MAX_LEN, ROUNDS = 20, 21, 10
BS = 160
NCORES = 8
PBS = [128, 32]
BOFF = [0, 128]
IL = 256
VP = 18 * 512
NEG = -30000.0
XR = [128, 128, 65]
K1 = K + 1          # qe contraction incl mask column
KI = K + 18         # psi contraction incl zero row + 16 indicators + const


def bcast_mid(ap, reps):
    return bass.AP(tensor=ap.tensor, offset=ap.offset,
                   ap=[ap.ap[0], [0, reps], ap.ap[1]])


def bcast_in(ap, reps):
    return bass.AP(tensor=ap.tensor, offset=ap.offset,
                   ap=[ap.ap[0], ap.ap[1], [0, reps]])


def regroup(ap, n, sz):
    return bass.AP(tensor=ap.tensor, offset=ap.offset,
                   ap=[ap.ap[0], [sz, n], [1, sz]])


def build_nc():
    nc = bacc.Bacc()

    def din(name, shape, dt):
        return nc.dram_tensor(name, shape, dt, kind="ExternalInput")

    w_gi = din("w_gi", [128, 11, 3 * H], BF16)   # tanh-form scaled
    w_gh = din("w_gh", [128, 4, 3 * H], BF16)
    bhh_n = din("bhh_n", [1, H], BF16)
    w_egi = din("w_egi", [128, 3, 3 * H], BF16)  # direct form
    w_egh = din("w_egh", [128, 4, 3 * H], BF16)
    ebhh_n = din("ebhh_n", [1, H], BF16)
    w_out = din("w_out", [128, 12, D], BF16)
    outb = din("outb", [1, D], BF16)
    w_qk = din("w_qk", [128, 4, K], BF16)
    w_qv = din("w_qv", [128, 4, H], BF16)
    w_ak = din("w_ak", [128, 4, K], BF16)
    akb = din("akb", [1, K], BF16)
    w_ik = din("w_ik", [128, 2, K], BF16)
    w_iv = din("w_iv", [128, 2, H], BF16)
    ivb = din("ivb", [1, H], BF16)
    qvb_c = din("qvb_c", [128, 4], F32)
    img_t = din("img_t", [128, 2, IL], BF16)
    emb_bf = din("emb_bf", [V, D], BF16)
    emb_aug = din("emb_aug", [V, D + 1], F32)
    embt_bf = din("embt_bf", [128, 3, VP], BF16)
    q_idx = din("q_idx", [128, 2 * L], U32)
    a_idx = din("a_idx", [128, 2 * L], U32)
    qe_mask = din("qe_mask", [128, 2, L], F32)
    am_h = din("am_h", [128, BS], BF16)
    ike_h = din("ike_h", [128, IL], BF16)
    ie_mask = din("ie_mask", [128, 2, IL], F32)

    out_o = nc.dram_tensor("out_o", [MAX_LEN, 3, 128, BS], F32,
                           kind="ExternalOutput")

    with tile.TileContext(nc) as tc:
        with (
            tc.tile_pool(name="cw", bufs=1) as cw,
            tc.tile_pool(name="pers", bufs=1) as pers,
            tc.tile_pool(name="wk", bufs=2) as wk,
            tc.tile_pool(name="st", bufs=1) as st,
            tc.tile_pool(name="pg", bufs=1, space="PSUM") as pgp,
            tc.tile_pool(name="ph", bufs=1, space="PSUM") as php,
        ):
            def load(pool, t, dt):
                s = pool.tile(list(t.shape), dt, name=t.name + "_sb")
                nc.sync.dma_start(s[:], t[:])
                return s

            s_qk = load(cw, w_qk, BF16)
            s_qv = load(cw, w_qv, BF16)
            s_ak = load(cw, w_ak, BF16)
            s_ik = load(cw, w_ik, BF16)
            s_iv = load(cw, w_iv, BF16)
            s_ivb = load(cw, ivb, BF16)
            s_imgt = load(cw, img_t, BF16)
            s_bhh = load(cw, bhh_n, BF16)
            s_ebhh = load(cw, ebhh_n, BF16)
            s_outb = load(cw, outb, BF16)
            s_akb = load(cw, akb, BF16)
            s_qvb = load(cw, qvb_c, F32)
            s_qidx = load(cw, q_idx, U32)
            s_aidx = load(cw, a_idx, U32)
            s_qem = load(cw, qe_mask, F32)
            s_iem = load(cw, ie_mask, F32)

            ident_bf = cw.tile([128, 128], BF16)
            make_identity(nc, ident_bf[:])
            ident_f32 = cw.tile([128, 128], F32)
            make_identity(nc, ident_f32[:])
            ones_bf = cw.tile([1, 192], BF16)
            nc.vector.memset(ones_bf[:], 1.0)
            sid4 = cw.tile([128, 32], BF16)
            for g4 in range(4):
                nc.vector.tensor_copy(sid4[32 * g4:32 * (g4 + 1), :],
                                      ident_bf[0:32, 0:32])
            iota8 = cw.tile([128, 8], F32)
            nc.gpsimd.iota(iota8[:], pattern=[[1, 8]], base=0, channel_multiplier=0,
                           allow_small_or_imprecise_dtypes=True)

            PG = pgp.tile([128, 4, 512], F32, name="PG")
            PH = php.tile([128, 4, 512], F32, name="PH")

            def trp_slot(bank, lo, hi):
                return PH[:, bank, lo:hi].bitcast(BF16)

            hD = pers.tile([128, 4, BS], BF16, name="hD")
            qk_b0 = pers.tile([128, L, K1], BF16)
            qk_b1 = pers.tile([128, L, K1], BF16)
            qkbs = [qk_b0, qk_b1]
            qv_b0 = pers.tile([128, L, H], BF16)
            qv_p1 = pers.tile([128, 5, H], BF16)
            ivv = pers.tile([128, 2, H], BF16)
            ikt = pers.tile([128, IL], BF16)
            nc.sync.dma_start(ikt[:], ike_h[:])
            qcT = pers.tile([128, 4, BS], BF16)
            icT = pers.tile([128, 4, BS], BF16)
            dec20 = pers.tile([128, 3, BS], BF16)

            nc.vector.memset(hD[:], 0.0)
            nc.vector.memset(dec20[32:64, 2, :], 0.0)
            nc.vector.memset(dec20[64:65, 2, :], 1.0)

            def tr(dst_sb_ap, src_sb_ap, pb, w, pt_ap, eng=None):
                nc.tensor.transpose(pt_ap[:w, :pb], src_sb_ap,
                                    ident_bf[:pb, :pb])
                (eng or nc.vector).tensor_copy(dst_sb_ap, pt_ap[:w, :pb])

            def tr_add(dst_sb_ap, src_sb_ap, bias_ap, pb, w, pt_ap):
                nc.tensor.transpose(pt_ap[:w, :pb], src_sb_ap,
                                    ident_bf[:pb, :pb])
                nc.vector.tensor_scalar_add(dst_sb_ap, pt_ap[:w, :pb], bias_ap)

            def fetch_x(idx_sb, t, slots):
                xt = wk.tile([128, 3, BS], BF16, tag="xt", bufs=4, name="xt")
                nc.vector.memset(xt[32:64, 2, :], 0.0)
                nc.vector.memset(xt[64:65, 2, :], 1.0)
                i = 0
                for c, (pb, off) in enumerate(zip(PBS, BOFF)):
                    g = wk.tile([128, D], BF16, tag="gath", bufs=4, name="g")
                    nc.gpsimd.indirect_dma_start(
                        out=g[:pb], out_offset=None, in_=emb_bf[:],
                        in_offset=bass.IndirectOffsetOnAxis(
                            ap=idx_sb[:pb, 2 * t + c:2 * t + c + 1], axis=0))
                    for kt in range(3):
                        w = 128 if kt < 2 else D - 256
                        tr(xt[:w, kt, off:off + pb], g[:pb, kt * 128:kt * 128 + w],
                           pb, w, slots[i % len(slots)])
                        i += 1
                return xt

            def emit_group(ps_ap, pairs):
                n = len(pairs)
                for i, (lh, rh) in enumerate(pairs):
                    nc.tensor.matmul(ps_ap, lh, rh, start=(i == 0), stop=(i == n - 1))

            def xh_round(wt, gbase, xt, dst, sl0):
                for j in range(4):
                    gc = slice(gbase + j * 128, gbase + (j + 1) * 128)
                    pairs = [(wt[:XR[kt], kt, gc], xt[:XR[kt], kt, :])
                             for kt in range(3)]
                    emit_group(PG[:, j, sl0:sl0 + BS], pairs)
                nc.vector.tensor_copy(dst[:], PG[:, :, sl0:sl0 + BS])

            # ---------- image projections ----------
            for mt in range(2):
                psv = PG[:, mt, 0:512]
                emit_group(psv, [(s_imgt[:, kt, mt * 128:(mt + 1) * 128],
                                  s_iv[:, kt, :]) for kt in range(2)])
                nc.scalar.copy(ivv[:, mt, :], psv)
            psik = PG[:K, 2, 0:IL]
            emit_group(psik, [(s_ik[:, kt, :K], s_imgt[:, kt, :])
                              for kt in range(2)])
            nc.vector.tensor_copy(ikt[:K, :], psik)

            # =================== encoder ===================
            with tc.tile_pool(name="qp", bufs=1) as qp:
                s_egi = load(qp, w_egi, BF16)
                s_egh = load(qp, w_egh, BF16)
                hE = qp.tile([128, 4, BS], BF16, name="hE")
                nc.vector.memset(hE[:], 0.0)
                xh_er = qp.tile([128, 3, 4, BS], BF16, name="xh_er")
                xh_ez = qp.tile([128, 3, 4, BS], BF16, name="xh_ez")
                xh_en = qp.tile([128, 3, 4, BS], F32, name="xh_en")
                enc_fsl = [trp_slot(0, 256, 336), trp_slot(1, 256, 336)]

                def enc_xh(tt):
                    xt = fetch_x(s_qidx, tt, enc_fsl)
                    r3 = tt % 3
                    xh_round(s_egi, 0, xt, xh_er[:, r3], 320)
                    xh_round(s_egi, H, xt, xh_ez[:, r3], 320)
                    xh_round(s_egi, 2 * H, xt, xh_en[:, r3], 320)

                enc_xh(0)
                enc_xh(1)
                for t in range(L):
                    r3 = t % 3
                    for j in range(4):
                        mc = slice(j * 128, (j + 1) * 128)
                        zc = slice(H + j * 128, H + (j + 1) * 128)
                        nxc = slice(2 * H + j * 128, 2 * H + (j + 1) * 128)
                        pairs = [(ident_bf[:], xh_er[:, r3, j, :])]
                        pairs += [(s_egh[:, kt, mc], hE[:, kt, :]) for kt in range(4)]
                        emit_group(PG[:, j, 0:BS], pairs)
                        pairs = [(ident_bf[:], xh_ez[:, r3, j, :])]
                        pairs += [(s_egh[:, kt, zc], hE[:, kt, :]) for kt in range(4)]
                        emit_group(PG[:, j, 160:160 + BS], pairs)
                        pairs = [(s_egh[:, kt, nxc], hE[:, kt, :]) for kt in range(4)]
                        pairs.append((s_ebhh[:, j * 128:(j + 1) * 128],
                                      ones_bf[:, :BS]))
                        emit_group(PH[:, j, 0:BS], pairs)
                    if t + 2 < L:
                        enc_xh(t + 2)
                    rz = st.tile([128, 4, 320], F32, tag="rz", name="rz")
                    nc.scalar.activation(rz[:], PG[:, :, 0:320], AF.Sigmoid)
                    t1 = st.tile([128, 4, BS], F32, tag="t1", name="t1")
                    nc.gpsimd.tensor_mul(t1[:], rz[:, :, 0:BS], PH[:, :, 0:BS])
                    cc = st.tile([128, 4, BS], F32, tag="cc", name="cc")
                    nc.vector.tensor_add(cc[:], t1[:], xh_en[:, r3])
                    n4 = st.tile([128, 4, BS], F32, tag="n4", name="n4")
                    nc.scalar.activation(n4[:], cc[:], AF.Tanh)
                    d4 = st.tile([128, 4, BS], F32, tag="d4", name="d4")
                    nc.gpsimd.tensor_sub(d4[:], hE[:], n4[:])
                    e4 = st.tile([128, 4, BS], F32, tag="e4", name="e4")
                    nc.gpsimd.tensor_mul(e4[:], rz[:, :, 160:160 + BS], d4[:])
                    nc.gpsimd.tensor_add(hE[:], n4[:], e4[:])
                    for bt, (pb, off) in enumerate(zip(PBS, BOFF)):
                        psk = PH[:pb, bt, 160:160 + K]
                        emit_group(psk, [(hE[:, kt, off:off + pb], s_qk[:, kt, :])
                                         for kt in range(4)])
                        nc.scalar.copy(qkbs[bt][:pb, t, 0:K], psk)
                        psv = PH[:pb, 2 + bt, 0:512]
                        emit_group(psv, [(hE[:, kt, off:off + pb], s_qv[:, kt, :])
                                         for kt in range(4)])
                        if bt == 0:
                            nc.scalar.copy(qv_b0[:pb, t, :], psv)
                        else:
                            g4 = t % 4
                            nc.scalar.copy(qv_p1[32 * g4:32 * (g4 + 1), t // 4, :],
                                           psv)

            # =================== decoder ===================
            with tc.tile_pool(name="lg", bufs=1) as lg:
                s_gi = load(lg, w_gi, BF16)
                s_gh = load(lg, w_gh, BF16)
                s_out = load(lg, w_out, BF16)
                o19T = lg.tile([128, 3, BS], BF16)
                nc.vector.memset(o19T[32:64, 2, :], 0.0)
                nc.vector.memset(o19T[64:65, 2, :], 1.0)
                o19_0 = lg.tile([128, D], F32)
                o19_1 = lg.tile([128, D], F32)
                o19_sb = [o19_0, o19_1]
                logit_sb = lg.tile([128, 8840], BF16)
                xh_r = lg.tile([128, 3, 4, BS], BF16, name="xh_r")
                xh_z = lg.tile([128, 3, 4, BS], BF16, name="xh_z")
                xh_n = lg.tile([128, 3, 4, BS], BF16, name="xh_n")
                dec_fsl = [trp_slot(2, 80, 160), trp_slot(3, 80, 160)]

                for c, (pb, off) in enumerate(zip(PBS, BOFF)):
                    nc.vector.tensor_copy(qkbs[c][:pb, :, K:K1],
                                          s_qem[:pb, c, :].unsqueeze(-1))
                aT_sb = lg.tile([128, BS], BF16, name="aT_pers")
                nc.sync.dma_start(aT_sb[:], am_h[:])

                def dec_xh(tt):
                    xt = fetch_x(s_aidx, tt, dec_fsl)
                    r3 = tt % 3
                    xh_round(s_gi, 0, xt, xh_r[:, r3], 160)
                    xh_round(s_gi, H, xt, xh_z[:, r3], 160)
                    xh_round(s_gi, 2 * H, xt, xh_n[:, r3], 160)

                xt_cur = fetch_x(s_aidx, 0, dec_fsl)
                for t in range(MAX_LEN):
                    r3 = t % 3
                    use_dec20 = (t == MAX_LEN - 1)
                    pa = PH[:K, 0, 160:160 + BS]
                    pairs = [(s_ak[:, kt, :], hD[:, kt, :]) for kt in range(4)]
                    pairs.append((s_akb[:], ones_bf[:, :BS]))
                    emit_group(pa, pairs)
                    nc.vector.tensor_copy(aT_sb[:K, :], pa)
                    for j in range(4):
                        nxc = slice(2 * H + j * 128, 2 * H + (j + 1) * 128)
                        pairs = [(s_gh[:, kt, nxc], hD[:, kt, :]) for kt in range(4)]
                        pairs.append((s_bhh[:, j * 128:(j + 1) * 128],
                                      ones_bf[:, :BS]))
                        if j < 3:
                            emit_group(PH[:, 1, 160 * j:160 * j + BS], pairs)
                        else:
                            emit_group(PH[:, 0, 320:320 + BS], pairs)
                    nh_sb = st.tile([128, 4, BS], F32, tag="nh", name="nh_sb")
                    nc.vector.tensor_copy(nh_sb[:, 0:3, :],
                                          regroup(PH[:, 1, 0:480], 3, BS))
                    nc.vector.tensor_copy(nh_sb[:, 3, :], PH[:, 0, 320:320 + BS])

                    a_b = st.tile([128, 2, K1], BF16, tag="ab", name="a_b")
                    for c, (pb, off) in enumerate(zip(PBS, BOFF)):
                        tr(a_b[:pb, c, :], aT_sb[:K1, off:off + pb], K1, pb,
                           trp_slot(2, 0, 80), eng=nc.gpsimd)
                    psis = []
                    for c, (pb, off) in enumerate(zip(PBS, BOFF)):
                        psi = PH[:pb, 2 + c, 160:160 + IL]
                        nc.tensor.matmul(psi, aT_sb[:KI, off:off + pb],
                                         ikt[:KI, :], start=True, stop=True)
                        psis.append(psi)

                    qw_bf = st.tile([128, 2, L], BF16, tag="qw", name="qw_bf")
                    for c, (pb, off) in enumerate(zip(PBS, BOFF)):
                        eng = nc.vector if c == 0 else nc.gpsimd
                        prod = wk.tile([128, L, K1], BF16, tag="prod", bufs=1,
                                       name="prod")
                        eng.tensor_mul(prod[:pb], qkbs[c][:pb],
                                       bcast_mid(a_b[:pb, c, :], L))
                        qe = st.tile([128, L], F32, tag="qe" + str(c), name="qe")
                        nc.vector.tensor_reduce(qe[:pb], prod[:pb], axis=AX.X,
                                                op=ALU.add)
                        ew = st.tile([128, L], F32, tag="ew" + str(c), name="ew")
                        ssum = st.tile([128, 1], F32, tag="ss" + str(c), name="ssum")
                        nc.scalar.activation(ew[:pb], qe[:pb], AF.Exp,
                                             scale=1.0, accum_out=ssum[:pb])
                        rs = st.tile([128, 1], F32, tag="rs" + str(c), name="rs")
                        nc.vector.reciprocal(rs[:pb], ssum[:pb])
                        nc.vector.tensor_scalar_mul(qw_bf[:pb, c, :], ew[:pb],
                                                    rs[:pb])

                    iwT = st.tile([128, 2, BS], BF16, tag="iwT", name="iwT")
                    for c, (pb, off) in enumerate(zip(PBS, BOFF)):
                        eng = nc.vector if c == 0 else nc.gpsimd
                        iem = st.tile([128, IL], F32, tag="iem" + str(c), name="iem")
                        eng.tensor_add(iem[:pb], psis[c], s_iem[:pb, c, :])
                        ewi = st.tile([128, IL], F32, tag="ewi" + str(c), name="ewi")
                        ssi = st.tile([128, 1], F32, tag="ssi" + str(c), name="ssi")
                        nc.scalar.activation(ewi[:pb], iem[:pb], AF.Exp,
                                             scale=1.0, accum_out=ssi[:pb])
                        rsi = st.tile([128, 1], F32, tag="rsi" + str(c), name="rsi")
                        nc.vector.reciprocal(rsi[:pb], ssi[:pb])
                        iwb = st.tile([128, IL], BF16, tag="iwb" + str(c),
                                      name="iwb")
                        eng.tensor_scalar_mul(iwb[:pb], ewi[:pb], rsi[:pb])
                        for cc2 in range(2):
                            tr(iwT[:, cc2, off:off + pb],
                               iwb[:pb, cc2 * 128:(cc2 + 1) * 128], pb, 128,
                               trp_slot(3, 0, 80))

                    ic_slots = [PH[:, 0, 160:160 + BS], PH[:, 1, 0:BS],
                                PH[:, 1, 160:160 + BS], PH[:, 1, 320:320 + BS]]
                    for j in range(4):
                        pairs = [(ivv[:, kt, j * 128:(j + 1) * 128],
                                  iwT[:, kt, :]) for kt in range(2)]
                        pairs.append((s_ivb[:, j * 128:(j + 1) * 128],
                                      ones_bf[:, :BS]))
                        emit_group(ic_slots[j], pairs)
                    nc.vector.tensor_copy(icT[:, 0, :], PH[:, 0, 160:160 + BS])
                    nc.vector.tensor_copy(icT[:, 1:4, :],
                                          regroup(PH[:, 1, 0:480], 3, BS))

                    qcb = st.tile([128, 2, 512], BF16, tag="qcb", name="qcb")
                    dg = wk.tile([128, L, 128], BF16, tag="diag", bufs=1, name="dg")
                    hl = L // 2
                    ibh = ident_bf[:128, :128]
                    ident_h = bass.AP(tensor=ibh.tensor, offset=ibh.offset,
                                      ap=[ibh.ap[0], [0, hl], ibh.ap[1]])
                    nc.gpsimd.tensor_mul(dg[:, :hl, :],
                                         bcast_in(qw_bf[:, 0, :hl], 128), ident_h)
                    nc.vector.tensor_mul(dg[:, hl:, :],
                                         bcast_in(qw_bf[:, 0, hl:], 128), ident_h)
                    psqc0 = PH[:, 2, 0:512]
                    for l in range(L):
                        nc.tensor.matmul(psqc0, dg[:, l, :], qv_b0[:, l, :],
                                         start=(l == 0), stop=(l == L - 1))
                    nc.scalar.copy(qcb[:, 0, :], psqc0)
                    qw_pk = st.tile([128, 5], BF16, tag="qpk", name="qw_pk")
                    for g4 in range(4):
                        nc.vector.tensor_copy(qw_pk[32 * g4:32 * (g4 + 1), :],
                                              qw_bf[0:32, 1, g4:L:4])
                    dg1 = wk.tile([128, 5, 32], BF16, tag="dg1", name="dg1")
                    sid_b = bass.AP(tensor=sid4.tensor, offset=sid4[:, :].offset,
                                    ap=[sid4[:, :].ap[0], [0, 5], sid4[:, :].ap[1]])
                    nc.vector.tensor_mul(dg1[:, :, :], bcast_in(qw_pk[:, :], 32),
                                         sid_b)
                    psqc1 = PH[:32, 3, 0:512]
                    for c5 in range(5):
                        nc.tensor.matmul(psqc1, dg1[:, c5, :], qv_p1[:, c5, :],
                                         start=(c5 == 0), stop=(c5 == 4))
                    nc.scalar.copy(qcb[:32, 1, :], psqc1)
                    for c, (pb, off) in enumerate(zip(PBS, BOFF)):
                        for kt in range(4):
                            tr_add(qcT[:, kt, off:off + pb],
                                   qcb[:pb, c, kt * 128:(kt + 1) * 128],
                                   s_qvb[:, kt:kt + 1], pb, 128,
                                   trp_slot(2, 0, 80) if c == 0
                                   else trp_slot(3, 0, 80))

                    # late gate groups
                    for j in range(4):
                        mc = slice(j * 128, (j + 1) * 128)
                        if use_dec20:
                            pairs = [(s_gi[:XR[kt], kt, mc], dec20[:XR[kt], kt, :])
                                     for kt in range(3)]
                        else:
                            pairs = [(s_gi[:XR[kt], kt, mc], xt_cur[:XR[kt], kt, :])
                                     for kt in range(3)]
                        pairs += [(s_gh[:, kt, mc], hD[:, kt, :]) for kt in range(4)]
                        pairs += [(s_gi[:, 3 + k, mc], qcT[:, k, :]) for k in range(4)]
                        pairs += [(s_gi[:, 7 + k, mc], icT[:, k, :]) for k in range(4)]
                        emit_group(PG[:, j, 0:BS], pairs)
                    for j in range(4):
                        zc = slice(H + j * 128, H + (j + 1) * 128)
                        if use_dec20:
                            pairs = [(s_gi[:XR[kt], kt, zc], dec20[:XR[kt], kt, :])
                                     for kt in range(3)]
                        else:
                            pairs = [(s_gi[:XR[kt], kt, zc], xt_cur[:XR[kt], kt, :])
                                     for kt in range(3)]
                        pairs += [(s_gh[:, kt, zc], hD[:, kt, :]) for kt in range(4)]
                        pairs += [(s_gi[:, 3 + k, zc], qcT[:, k, :]) for k in range(4)]
                        pairs += [(s_gi[:, 7 + k, zc], icT[:, k, :]) for k in range(4)]
                        emit_group(PH[:, j, 0:BS], pairs)
                    for j in range(4):
                        nxc = slice(2 * H + j * 128, 2 * H + (j + 1) * 128)
                        if use_dec20:
                            pairs = [(s_gi[:XR[kt], kt, nxc],
                                      dec20[:XR[kt], kt, :]) for kt in range(3)]
                        else:
                            pairs = [(s_gi[:XR[kt], kt, nxc], xt_cur[:XR[kt], kt, :])
                                     for kt in range(3)]
                        pairs += [(s_gi[:, 3 + k, nxc], qcT[:, k, :]) for k in range(4)]
                        pairs += [(s_gi[:, 7 + k, nxc], icT[:, k, :]) for k in range(4)]
                        emit_group(PG[:, j, 320:320 + BS], pairs)

                    if t + 1 < L:
                        xt_nxt = fetch_x(s_aidx, t + 1, dec_fsl)

                    trz = st.tile([128, 4, BS], F32, tag="trz", name="trz")
                    tzz = st.tile([128, 4, BS], F32, tag="tzz", name="tzz")
                    nc.scalar.activation(trz[:], PG[:, :, 0:BS], AF.Tanh)
                    nc.scalar.activation(tzz[:], PH[:, :, 0:BS], AF.Tanh)
                    aa = st.tile([128, 4, BS], F32, tag="aa", name="aa")
                    nc.gpsimd.tensor_mul(aa[:], trz[:], nh_sb[:])
                    bb = st.tile([128, 4, BS], F32, tag="bb", name="bb")
                    nc.gpsimd.tensor_add(bb[:], aa[:], nh_sb[:])
                    cc = st.tile([128, 4, BS], F32, tag="cc", name="cc")
                    nc.vector.tensor_add(cc[:], bb[:], PG[:, :, 320:320 + BS])
                    n4 = st.tile([128, 4, BS], F32, tag="n4", name="n4")
                    nc.scalar.activation(n4[:], cc[:], AF.Tanh, scale=0.5)
                    z4 = st.tile([128, 4, BS], F32, tag="z4", name="z4")
                    nc.vector.tensor_scalar(out=z4[:], in0=tzz[:], scalar1=0.5,
                                            scalar2=0.5, op0=ALU.mult,
                                            op1=ALU.add)
                    d4 = st.tile([128, 4, BS], F32, tag="d4", name="d4")
                    nc.gpsimd.tensor_sub(d4[:], hD[:], n4[:])
                    e4 = st.tile([128, 4, BS], F32, tag="e4", name="e4")
                    nc.gpsimd.tensor_mul(e4[:], z4[:], d4[:])
                    nc.gpsimd.tensor_add(hD[:], n4[:], e4[:])

                    oT = st.tile([128, 3, BS], F32, tag="oT", bufs=2, name="oT")
                    po_slots = [PH[:, 2, 160:160 + BS], PH[:, 3, 160:160 + BS],
                                PH[:D - 256, 2, 320:320 + BS]]
                    for j in range(3):
                        w = 128 if j < 2 else D - 256
                        mc = slice(j * 128, j * 128 + w)
                        po = po_slots[j]
                        pairs = [(s_out[:, k, mc], hD[:, k, :]) for k in range(4)]
                        pairs += [(s_out[:, 4 + k, mc], qcT[:, k, :])
                                  for k in range(4)]
                        pairs += [(s_out[:, 8 + k, mc], icT[:, k, :])
                                  for k in range(4)]
                        pairs.append((s_outb[:, mc], ones_bf[:, :BS]))
                        emit_group(po, pairs)
                        nc.scalar.copy(oT[:w, j, :], po)
                        nc.sync.dma_start(out_o[t, j, :w, :], oT[:w, j, :])

                    if t + 1 < L:
                        xt_cur = xt_nxt
                    if t == MAX_LEN - 2:
                        nc.vector.tensor_copy(o19T[:, 0:2, :], oT[:, 0:2, :])
                        nc.vector.tensor_copy(o19T[:D - 256, 2, :],
                                              oT[:D - 256, 2, :])
                        pt32 = PH[:, 1, 160:288]
                        for c, (pb, off) in enumerate(zip(PBS, BOFF)):
                            for kt in range(3):
                                w = 128 if kt < 2 else D - 256
                                nc.tensor.transpose(pt32[:pb, :w],
                                                    oT[:w, kt, off:off + pb],
                                                    ident_f32[:w, :w])
                                nc.vector.tensor_copy(
                                    o19_sb[c][:pb, kt * 128:kt * 128 + w],
                                    pt32[:pb, :w])
                        for c, (pb, off) in enumerate(zip(PBS, BOFF)):
                            for nci in range(18):
                                ncw = 512 if nci < 17 else V - 17 * 512
                                rhs = wk.tile([128, 3, 512], BF16, tag="lrhs",
                                              bufs=2, name="rhs")
                                for kt in range(3):
                                    nr = 128 if kt < 2 else 65
                                    nc.sync.dma_start(
                                        rhs[:nr, kt, :ncw],
                                        embt_bf[:nr, kt,
                                                nci * 512:nci * 512 + ncw])
                                psl = PH[:pb, 2 + (nci % 2), 0:ncw]
                                pairs = []
                                for kt in range(3):
                                    nr = 128 if kt < 2 else 65
                                    pairs.append((o19T[:nr, kt, off:off + pb],
                                                  rhs[:nr, kt, :ncw]))
                                emit_group(psl, pairs)
                                nc.scalar.copy(
                                    logit_sb[:pb, nci * 512:nci * 512 + ncw],
                                    psl)
                            if c == 0:
                                nc.vector.memset(logit_sb[:, V:], -60000.0)
                            mx8 = st.tile([128, 8], BF16, name="mx8")
                            nc.vector.max(mx8[:pb], logit_sb[:pb])
                            ix8 = st.tile([128, 8], U32, name="ix8")
                            nc.vector.max_index(ix8[:pb], mx8[:pb], logit_sb[:pb])
                            scores = st.tile([128, 8], F32, name="scores")
                            for jj in range(8):
                                g8 = wk.tile([128, D + 1], F32, tag="gath8",
                                             name="g8")
                                nc.gpsimd.indirect_dma_start(
                                    out=g8[:pb], out_offset=None, in_=emb_aug[:],
                                    in_offset=bass.IndirectOffsetOnAxis(
                                        ap=ix8[:pb, jj:jj + 1], axis=0))
                                pr = wk.tile([128, D], F32, tag="pr8", name="pr")
                                nc.vector.tensor_mul(pr[:pb], o19_sb[c][:pb],
                                                     g8[:pb, :D])
                                sj = st.tile([128, 1], F32, name="sj")
                                nc.vector.tensor_reduce(sj[:pb], pr[:pb],
                                                        axis=AX.X, op=ALU.add)
                                nc.vector.tensor_add(scores[:pb, jj:jj + 1],
                                                     sj[:pb], g8[:pb, D:D + 1])
                            m1 = st.tile([128, 8], F32, name="m1")
                            nc.vector.max(m1[:pb], scores[:pb])
                            j1 = st.tile([128, 8], U32, name="j1")
                            nc.vector.max_index(j1[:pb], m1[:pb], scores[:pb])
                            j1f = st.tile([128, 1], F32, name="j1f")
                            nc.vector.tensor_copy(j1f[:pb], j1[:pb, 0:1])
                            oh = st.tile([128, 8], F32, name="oh")
                            nc.vector.tensor_scalar(out=oh[:pb], in0=iota8[:pb],
                                                    scalar1=j1f[:pb],
                                                    scalar2=None,
                                                    op0=ALU.is_equal)
                            ix8f = st.tile([128, 8], F32, name="ix8f")
                            nc.vector.tensor_copy(ix8f[:pb], ix8[:pb])
                            nc.vector.tensor_mul(ix8f[:pb], oh[:pb], ix8f[:pb])
                            vsum = st.tile([128, 1], F32, name="vsum")
                            nc.vector.tensor_reduce(vsum[:pb], ix8f[:pb],
                                                    axis=AX.X, op=ALU.add)
                            vidx = st.tile([128, 1], U32, name="vidx")
                            nc.vector.tensor_copy(vidx[:pb], vsum[:pb])
                            gm = wk.tile([128, D], BF16, tag="gath", bufs=4,
                                         name="gm")
                            nc.gpsimd.indirect_dma_start(
                                out=gm[:pb], out_offset=None, in_=emb_bf[:],
                                in_offset=bass.IndirectOffsetOnAxis(
                                    ap=vidx[:pb, 0:1], axis=0))
                            for kt in range(3):
                                w = 128 if kt < 2 else D - 256
                                tr(dec20[:w, kt, off:off + pb],
                                   gm[:pb, kt * 128:kt * 128 + w], pb, w,
                                   trp_slot(2, 0, 80))

    nc.compile()
    return nc


_NC_CACHE = None


def _get_nc():
    global _NC_CACHE
    if _NC_CACHE is None:
        _NC_CACHE = build_nc()
    return _NC_CACHE


def _pad_tiles(a, ntiles):
    rows, cols = a.shape
    out = np.zeros((128 * ntiles, cols), a.dtype)
    out[:rows] = a
    return np.ascontiguousarray(out.reshape(ntiles, 128, cols).transpose(1, 0, 2))


def _prep_shared(inputs):
    bf = np.float16
    f32 = np.float32
    eW = np.asarray(inputs["embed_W"], f32)
    d = {}
    wih = np.asarray(inputs["dec_W_ih"], f32)
    bih = np.asarray(inputs["dec_b_ih"], f32)
    bhh = np.asarray(inputs["dec_b_hh"], f32)
    gi = np.zeros((128 * 11, 3 * H), f32)
    gi[0:D] = wih[:, 0:D].T
    gi[320] = bih + np.concatenate([bhh[:2 * H], np.zeros(H, f32)])
    gi[384:384 + H] = wih[:, D:D + H].T
    gi[896:896 + H] = wih[:, D + H:].T
    gi[:, 0:2 * H] *= 0.5
    gi[:, 2 * H:] *= 2.0
    d["w_gi"] = _pad_tiles(gi.astype(bf), 11)
    gh = np.asarray(inputs["dec_W_hh"], f32).T.copy()
    gh[:, 0:2 * H] *= 0.5
    d["w_gh"] = _pad_tiles(gh.astype(bf), 4)
    d["bhh_n"] = np.ascontiguousarray(bhh[2 * H:].astype(bf)[None, :])
    ewih = np.asarray(inputs["enc_W_ih"], f32)
    ebih = np.asarray(inputs["enc_b_ih"], f32)
    ebhh = np.asarray(inputs["enc_b_hh"], f32)
    egi = np.zeros((128 * 3, 3 * H), f32)
    egi[0:D] = ewih[:, :D].T
    egi[320] = ebih + np.concatenate([ebhh[:2 * H], np.zeros(H, f32)])
    d["w_egi"] = _pad_tiles(egi.astype(bf), 3)
    d["w_egh"] = _pad_tiles(np.asarray(inputs["enc_W_hh"], f32).T.astype(bf), 4)
    d["ebhh_n"] = np.ascontiguousarray(ebhh[2 * H:].astype(bf)[None, :])
    d["w_out"] = _pad_tiles(np.asarray(inputs["out_W"], f32).T.astype(bf), 12)
    d["outb"] = np.ascontiguousarray(
        np.asarray(inputs["out_b"], f32).astype(bf)[None, :])
    d["w_qk"] = _pad_tiles(np.asarray(inputs["qk_W"], f32).T.astype(bf), 4)
    d["w_qv"] = _pad_tiles(np.asarray(inputs["qv_W"], f32).T.astype(bf), 4)
    d["qvb_c"] = np.ascontiguousarray(
        np.asarray(inputs["qv_b"], f32).reshape(4, 128).T)
    d["w_ak"] = _pad_tiles(np.asarray(inputs["ak_W"], f32).T.astype(bf), 4)
    d["akb"] = np.ascontiguousarray(
        np.asarray(inputs["ak_b"], f32).astype(bf)[None, :])
    d["w_ik"] = _pad_tiles(np.asarray(inputs["ik_W"], f32).T.astype(bf), 2)
    d["w_iv"] = _pad_tiles(np.asarray(inputs["iv_W"], f32).T.astype(bf), 2)
    d["ivb"] = np.ascontiguousarray(
        np.asarray(inputs["iv_b"], f32).astype(bf)[None, :])
    d["emb_bf"] = eW.astype(bf)
    wd_b = np.asarray(inputs["wd_b"], f32)
    d["emb_aug"] = np.ascontiguousarray(np.concatenate([eW, wd_b[:, None]], 1))
    aug = np.zeros((128 * 3, VP), f32)
    aug[:D, :V] = eW.T
    aug[320, :V] = wd_b
    d["embt_bf"] = _pad_tiles(aug.astype(bf), 3)
    am = np.zeros((128, BS), f32)
    am[K, :] = 1.0
    for b in range(BS):
        am[51 + b // ROUNDS, b] = 1.0
    am[67, :] = 1.0
    d["am_h"] = am.astype(bf)
    ike = np.zeros((128, IL), f32)
    for i in range(16):
        ike[51 + i, 16 * i:16 * (i + 1)] = -NEG
    ike[67, :] = NEG
    d["ike_h"] = ike.astype(bf)
    return d


def _idx_cols(seq_rows):
    out = np.zeros((128, 2 * L), np.uint32)
    for t in range(L):
        out[:, 2 * t] = seq_rows[0:128, t]
        out[:32, 2 * t + 1] = seq_rows[128:160, t]
    return out


def _build_maps(inputs, shared):
    f32 = np.float32
    bf = np.float16
    ques = np.asarray(inputs["ques_seqs"]).astype(np.uint32)
    ans = np.asarray(inputs["ans_seqs"]).astype(np.uint32)
    qlens = np.asarray(inputs["ques_lens"]).astype(np.int64)
    img = np.asarray(inputs["img_seqs"], f32)
    maps = []
    for s in range(NCORES):
        m = dict(shared)
        r0 = s * BS
        m["q_idx"] = _idx_cols(ques[r0:r0 + BS, :L])
        m["a_idx"] = _idx_cols(ans[r0:r0 + BS, :L])
        qm = np.full((128, 2, L), NEG, f32)
        lens = qlens[r0:r0 + BS]
        for bt, (pb, off) in enumerate(zip(PBS, BOFF)):
            for b in range(pb):
                qm[b, bt, :lens[off + b]] = 0.0
        m["qe_mask"] = qm
        im = np.full((128, 2, IL), NEG, f32)
        for bt, (pb, off) in enumerate(zip(PBS, BOFF)):
            for b in range(pb):
                gimg = (off + b) // ROUNDS
                im[b, bt, gimg * 16:(gimg + 1) * 16] = 0.0
        m["ie_mask"] = im
        imgs = img[s * 16:(s + 1) * 16].reshape(IL, 256)
        it = np.zeros((128 * 2, IL), f32)
        it[:256] = imgs.T
        m["img_t"] = np.ascontiguousarray(
            it.reshape(2, 128, IL).transpose(1, 0, 2)).astype(bf)
        maps.append(m)
    return maps


def kernel(**inputs):
    nc = _get_nc()
    shared = _prep_shared(inputs)
    in_maps = _build_maps(inputs, shared)
    from concourse.bass_utils import run_bass_kernel_spmd
    res = run_bass_kernel_spmd(nc, in_maps, core_ids=list(range(NCORES)))
    outs = []
    for s in range(NCORES):
        o = np.asarray(res.results[s]["out_o"])  # [21, 3, 128, 160]
        o = o.reshape(MAX_LEN, 3 * 128, BS)[:, :D, :]
        outs.append(np.ascontiguousarray(o.transpose(2, 0, 1)))
    return np.concatenate(outs, 0).astype(np.float32)
